# revision 6
# baseline (speedup 1.0000x reference)
"""Trainium2 Bass kernel for nn_AIGStateEncoder (2-layer LSTM + linear head).

Data-parallel over batch: B=4096 rows split across 8 NeuronCores (512 each).
Per core the two LSTM layers are fused into one recurrence ("combined step"
s runs layer0 at t=s and layer1 at t=s-1), with the state kept transposed
(hidden units on SBUF partitions, batch on the free dimension) and the two
layers stacked on the 128 partitions: [layer0 (0:64); layer1 (64:128)].
Two independent batch groups of 256 rows hide each other's serial chain.

ScalarE is the bottleneck engine (activation cost ~ (FD + 224c)/1.2GHz,
1x rate), so the design minimizes activation instruction count per step:

  - One state tile S = [h0; h1] (K=128) per group; each gate's preact is ONE
    matmul with a [128, 128] stationary whose columns 0:64 carry layer0's
    W_hh0 (zeros in the h1 rows) and columns 64:128 carry layer1's
    [W_ih1; W_hh1].  A second K=2 matmul accumulates x_t * W_ih0 + biases
    from a tiny [x_t; 1] moving tile, so no activation needs the bias port.
  - Gates i,f,o live in three consecutive PSUM banks (one [128, 3, 512]
    tile); a single SIGMOID activation with a strided [128, 3, 256] access
    pattern covers all three.  Gate g and tanh(c) are separate TANH ops.
  - Cell update on VectorE in bf16: t1=f*c, t2=i*g, c=t1+t2, S'=o*tanh(c)
    (one op writes both h0 and h1 halves).

Everything computes in bf16 (fp32 PSUM accumulation).
"""
import sys

if '/opt/trn_rl_repo' not in sys.path:
    sys.path.insert(0, '/opt/trn_rl_repo')

import numpy as np
import ml_dtypes

B, T, H = 4096, 256, 64
N_CORES = 8
B_LOC = B // N_CORES  # 512

GATES = ("i", "f", "o", "g")  # i,f,o merged sigmoid; g separate tanh
GCOL = {"i": 0, "f": 1, "g": 2, "o": 3}  # PyTorch gate order i,f,g,o


def _split_excess_waits(nc, limit=1):
    """The walrus build in this container accepts at most one sync wait per
    instruction.  Hoist excess waits onto NoOps inserted just before the
    instruction on the same engine (same-engine program order preserves the
    synchronization semantics)."""
    import concourse.mybir as mybir
    ctr = 0
    for f in nc.m.functions:
        for bb in f.blocks:
            il = bb.instructions
            i = 0
            while i < len(il):
                ins = il[i]
                si = ins.sync_info
                if si is not None and si.on_wait and len(si.on_wait) > limit:
                    waits = list(si.on_wait)
                    excess, keep = waits[:-limit], waits[-limit:]
                    while excess:
                        chunk, excess = excess[:limit], excess[limit:]
                        nop = mybir.InstNoOp(name=f"waitsplit_{ctr}", ins=[], outs=[])
                        ctr += 1
                        nop.engine = ins.engine
                        nop.sync_info = mybir.SyncInfo(on_wait=chunk, on_update=[])
                        il.insert(i, nop)
                        i += 1
                    ins.sync_info = mybir.SyncInfo(on_wait=keep,
                                                   on_update=list(si.on_update))
                i += 1


def _build_program(n_groups=2):
    import concourse.bass as bass
    import concourse.mybir as mybir
    from concourse.tile import TileContext

    BF16 = mybir.dt.bfloat16
    F32 = mybir.dt.float32
    AF = mybir.ActivationFunctionType
    OP = mybir.AluOpType

    NG = n_groups
    NB = B_LOC // NG

    nc = bass.Bass()
    # xT2: x transposed, duplicated row layout [T, B_LOC] (one DMA per step)
    xT = nc.declare_dram_parameter("xT", [T, B_LOC], BF16, isOutput=False)
    # per-gate big stationary [K=128, 4, 128] and x/bias stationary [K=2, 4, 128]
    wbig = nc.declare_dram_parameter("wbig", [128, 4 * 128], BF16, isOutput=False)
    wxb = nc.declare_dram_parameter("wxb", [2, 4 * 128], BF16, isOutput=False)
    wlin = nc.declare_dram_parameter("wlin", [H, H], BF16, isOutput=False)
    blin = nc.declare_dram_parameter("blin", [128, H], F32, isOutput=False)
    out = nc.declare_dram_parameter("out", [B_LOC, H], F32, isOutput=True)

    with TileContext(nc) as tc:
        with (
            tc.tile_pool(name="const", bufs=1) as cpool,
            tc.tile_pool(name="state", bufs=1) as spool,
            tc.tile_pool(name="work", bufs=3) as wpool,
            tc.tile_pool(name="psum", bufs=1, space="PSUM") as ppool,
        ):
            wbig_s = cpool.tile([128, 4, 128], BF16, tag="wbig", name="wbig")
            nc.sync.dma_start(wbig_s[:], wbig[:])
            wxb_s = cpool.tile([2, 4, 128], BF16, tag="wxb", name="wxb")
            nc.sync.dma_start(wxb_s[:], wxb[:])
            wlin_s = cpool.tile([H, H], BF16, tag="wlin", name="wlin")
            nc.sync.dma_start(wlin_s[:], wlin[:])
            blin_s = cpool.tile([128, H], F32, tag="blin", name="blin")
            nc.sync.dma_start(blin_s[:], blin[:])

            # State tiles: S = [h0 (0:64); h1 (64:128)], double-buffered.
            S = []
            c_st = []
            h1f = []
            for g in range(NG):
                S.append([spool.tile([2 * H, NB], BF16, tag=f"S{g}_{p}",
                                     name=f"S{g}_{p}") for p in range(2)])
                c_st.append(spool.tile([2 * H, NB], BF16, tag=f"c{g}", name=f"c{g}"))
                h1f.append(spool.tile([H, NB], BF16, tag=f"h1f{g}", name=f"h1f{g}"))
                for p in range(2):
                    nc.gpsimd.memset(S[g][p][:], 0.0)
                nc.gpsimd.memset(c_st[g][:], 0.0)
            # x/ones moving tiles: partition 0 = x_t (whole core batch),
            # partition 1 = ones.  Double-buffered; DMA writes row 0 only.
            X1 = [spool.tile([2, B_LOC], BF16, tag=f"X1_{p}", name=f"X1_{p}")
                  for p in range(2)]
            for p in range(2):
                nc.gpsimd.memset(X1[p][:], 1.0)
            nc.sync.dma_start(X1[0][0:1, :], xT[0:1, :])

            # PSUM: per group, one [128, 3, 512] tile = 3 banks (i,f,o; each
            # gate's data in cols 0:256 of its own bank) + one [128, 512]
            # bank for g.  2 groups x 4 banks = 8 banks.
            ps_ifo = [ppool.tile([2 * H, 3, 512], F32, tag=f"ps_ifo{g}",
                                 name=f"ps_ifo{g}") for g in range(NG)]
            ps_g = [ppool.tile([2 * H, 512], F32, tag=f"ps_g{g}",
                               name=f"ps_g{g}") for g in range(NG)]

            def gate_ps(g, gt):
                if gt == "g":
                    return ps_g[g][:, 0:NB]
                k = {"i": 0, "f": 1, "o": 2}[gt]
                return ps_ifo[g][:, k, 0:NB]

            def step(s):
                cur, nxt = s % 2, (s + 1) % 2
                l0 = s < T
                l1 = s >= 1
                lo = 0 if l0 else H
                hi = 2 * H if l1 else H
                for g in range(NG):
                    cols = slice(g * NB, (g + 1) * NB)
                    # 8 matmuls: per gate, big (K=128) + x/bias (K=2) pair.
                    for gt in GATES:
                        gi = GATES.index(gt)
                        ps = gate_ps(g, gt)
                        nc.tensor.matmul(ps, wbig_s[:, gi, :], S[g][cur][:],
                                         start=True, stop=False)
                        nc.tensor.matmul(ps, wxb_s[:, gi, :], X1[cur][:, cols],
                                         start=False, stop=True)
                    # Activations: one merged sigmoid (i,f,o), one tanh (g).
                    a = wpool.tile([2 * H, 3, NB], BF16, tag=f"a{g}", name=f"a{g}")
                    nc.scalar.activation(a[lo:hi, :, :],
                                         ps_ifo[g][lo:hi, :, 0:NB], AF.Sigmoid)
                    ag = wpool.tile([2 * H, NB], BF16, tag=f"ag{g}", name=f"ag{g}")
                    nc.scalar.activation(ag[lo:hi, :], ps_g[g][lo:hi, 0:NB], AF.Tanh)
                    # Cell update (VectorE, bf16).
                    t1 = wpool.tile([2 * H, NB], BF16, tag=f"t1{g}", name=f"t1{g}")
                    nc.vector.tensor_tensor(t1[lo:hi, :], a[lo:hi, 1, :],
                                            c_st[g][lo:hi, :], op=OP.mult)
                    t2 = wpool.tile([2 * H, NB], BF16, tag=f"t2{g}", name=f"t2{g}")
                    nc.vector.tensor_tensor(t2[lo:hi, :], a[lo:hi, 0, :],
                                            ag[lo:hi, :], op=OP.mult)
                    nc.vector.tensor_tensor(c_st[g][lo:hi, :], t1[lo:hi, :],
                                            t2[lo:hi, :], op=OP.add)
                    tC = wpool.tile([2 * H, NB], BF16, tag=f"tC{g}", name=f"tC{g}")
                    nc.scalar.activation(tC[lo:hi, :], c_st[g][lo:hi, :], AF.Tanh)
                    if s < T:
                        nc.vector.tensor_tensor(S[g][nxt][lo:hi, :],
                                                a[lo:hi, 2, :], tC[lo:hi, :],
                                                op=OP.mult)
                    else:
                        nc.vector.tensor_tensor(h1f[g][:], a[H:2 * H, 2, :],
                                                tC[H:2 * H, :], op=OP.mult)
                if s + 1 < T:
                    nc.sync.dma_start(X1[nxt][0:1, :], xT[s + 1:s + 2, :])

            for s in range(T + 1):
                step(s)

            # final linear: out[b, :] = h1f.T @ wlin + blin
            for g in range(NG):
                for blk in range(NB // 128):
                    psl = ppool.tile([128, H], F32, tag="ps_ifo0", name="psl")
                    nc.tensor.matmul(psl[:], h1f[g][:, blk * 128:(blk + 1) * 128],
                                     wlin_s[:], start=True, stop=True)
                    ob = wpool.tile([128, H], F32, tag="ob", name="ob")
                    nc.vector.scalar_tensor_tensor(ob[:], psl[:], 1.0,
                                                   blin_s[:],
                                                   op0=OP.mult, op1=OP.add)
                    row0 = g * NB + blk * 128
                    nc.sync.dma_start(out[row0:row0 + 128, :], ob[:])

    _split_excess_waits(nc, limit=1)
    return nc


def _prep_inputs(inputs):
    bf = ml_dtypes.bfloat16
    f32 = np.float32
    recipe = np.ascontiguousarray(np.asarray(inputs["recipe"], f32).reshape(B, T))
    W_hh0 = np.asarray(inputs["W_hh0"], f32)   # [256, 64]
    W_ih0 = np.asarray(inputs["W_ih0"], f32)   # [256, 1]
    W_ih1 = np.asarray(inputs["W_ih1"], f32)   # [256, 64]
    W_hh1 = np.asarray(inputs["W_hh1"], f32)   # [256, 64]
    b0 = np.asarray(inputs["b_ih0"], f32) + np.asarray(inputs["b_hh0"], f32)
    b1 = np.asarray(inputs["b_ih1"], f32) + np.asarray(inputs["b_hh1"], f32)

    # Per-gate big stationary [K=128, M=128]:
    #   cols 0:64 (layer0 out rows): rows 0:64 = W_hh0.T, rows 64:128 = 0
    #   cols 64:128 (layer1 out rows): rows 0:64 = W_ih1.T, 64:128 = W_hh1.T
    wbig = np.zeros((4, 128, 128), f32)
    wxb = np.zeros((4, 2, 128), f32)
    for gi, gt in enumerate(GATES):
        rows = slice(GCOL[gt] * H, (GCOL[gt] + 1) * H)
        wbig[gi, 0:64, 0:64] = W_hh0[rows, :].T
        wbig[gi, 0:64, 64:128] = W_ih1[rows, :].T
        wbig[gi, 64:128, 64:128] = W_hh1[rows, :].T
        wxb[gi, 0, 0:64] = W_ih0[rows, 0]
        wxb[gi, 1, 0:64] = b0[rows]
        wxb[gi, 1, 64:128] = b1[rows]

    wlin = np.ascontiguousarray(np.asarray(inputs["W_lin"], f32).T).astype(bf)
    blin = np.tile(np.asarray(inputs["b_lin"], f32), (128, 1))
    # SBUF layout: partition dim = K first -> [K, gate, M]
    wbig2 = np.ascontiguousarray(wbig.transpose(1, 0, 2)).reshape(128, 512).astype(bf)
    wxb2 = np.ascontiguousarray(wxb.transpose(1, 0, 2)).reshape(2, 512).astype(bf)

    in_maps = []
    for i in range(N_CORES):
        shard = recipe[i * B_LOC:(i + 1) * B_LOC]
        xTs = np.ascontiguousarray(shard.T).astype(bf)
        in_maps.append({"xT": xTs, "wbig": wbig2, "wxb": wxb2,
                        "wlin": wlin, "blin": blin})
    return in_maps


_PROGRAM = []


def _run(inputs, trace=False):
    from concourse.bass_utils import run_bass_kernel_spmd
    if not _PROGRAM:
        _PROGRAM.append(_build_program())
    nc = _PROGRAM[0]
    in_maps = _prep_inputs(inputs)
    last_err = None
    for attempt in range(3):
        try:
            res = run_bass_kernel_spmd(nc, in_maps,
                                       core_ids=list(range(N_CORES)), trace=trace)
            outs = [np.asarray(res.results[i]["out"]) for i in range(N_CORES)]
            return np.concatenate(outs, axis=0), res
        except Exception as e:  # transient first-exec device faults: retry
            last_err = e
    raise last_err


def kernel(**inputs):
    full, _ = _run(inputs, trace=False)
    return full.astype(np.float32)


# revision 7
# speedup vs baseline: 1.2202x; 1.2202x over previous
"""Trainium2 Bass kernel for nn_AIGStateEncoder (2-layer LSTM + linear head).

Data-parallel over batch: B=4096 rows split across 8 NeuronCores (512 each).
Per core the two LSTM layers are fused into one recurrence ("combined step"
s runs layer0 at t=s and layer1 at t=s-1), with the state kept transposed
(hidden units on SBUF partitions, batch on the free dimension) and the two
layers stacked on the 128 partitions: [layer0 (0:64); layer1 (64:128)].
Two independent batch groups of 256 rows hide each other's serial chain.

ScalarE is the bottleneck engine (activation cost ~ (FD + 224c)/1.2GHz,
1x rate), so the design minimizes activation instruction count per step:

  - One state tile S = [h0; h1] (K=128) per group; each gate's preact is ONE
    matmul with a [128, 128] stationary whose columns 0:64 carry layer0's
    W_hh0 (zeros in the h1 rows) and columns 64:128 carry layer1's
    [W_ih1; W_hh1].  A second K=2 matmul accumulates x_t * W_ih0 + biases
    from a tiny [x_t; 1] moving tile, so no activation needs the bias port.
  - Gates i,f,o live in three consecutive PSUM banks (one [128, 3, 512]
    tile); a single SIGMOID activation with a strided [128, 3, 256] access
    pattern covers all three.  Gate g and tanh(c) are separate TANH ops.
  - Cell update on VectorE in bf16: t1=f*c, t2=i*g, c=t1+t2, S'=o*tanh(c)
    (one op writes both h0 and h1 halves).

Everything computes in bf16 (fp32 PSUM accumulation).
"""
import sys

if '/opt/trn_rl_repo' not in sys.path:
    sys.path.insert(0, '/opt/trn_rl_repo')

import numpy as np
import ml_dtypes

B, T, H = 4096, 256, 64
N_CORES = 8
B_LOC = B // N_CORES  # 512

GATES = ("i", "f", "o", "g")  # i,f,o merged sigmoid; g separate tanh
GCOL = {"i": 0, "f": 1, "g": 2, "o": 3}  # PyTorch gate order i,f,g,o


def _split_excess_waits(nc, limit=1):
    """The walrus build in this container accepts at most one sync wait per
    instruction.  Hoist excess waits onto NoOps inserted just before the
    instruction on the same engine (same-engine program order preserves the
    synchronization semantics)."""
    import concourse.mybir as mybir
    ctr = 0
    for f in nc.m.functions:
        for bb in f.blocks:
            il = bb.instructions
            i = 0
            while i < len(il):
                ins = il[i]
                si = ins.sync_info
                if si is not None and si.on_wait and len(si.on_wait) > limit:
                    waits = list(si.on_wait)
                    excess, keep = waits[:-limit], waits[-limit:]
                    while excess:
                        chunk, excess = excess[:limit], excess[limit:]
                        nop = mybir.InstNoOp(name=f"waitsplit_{ctr}", ins=[], outs=[])
                        ctr += 1
                        nop.engine = ins.engine
                        nop.sync_info = mybir.SyncInfo(on_wait=chunk, on_update=[])
                        il.insert(i, nop)
                        i += 1
                    ins.sync_info = mybir.SyncInfo(on_wait=keep,
                                                   on_update=list(si.on_update))
                i += 1


def _build_program(n_groups=2):
    import concourse.bass as bass
    import concourse.mybir as mybir
    from concourse.tile import TileContext

    BF16 = mybir.dt.bfloat16
    F32 = mybir.dt.float32
    AF = mybir.ActivationFunctionType
    OP = mybir.AluOpType

    NG = n_groups
    NB = B_LOC // NG

    nc = bass.Bass()
    # xT2: x transposed, duplicated row layout [T, B_LOC] (one DMA per step)
    xT = nc.declare_dram_parameter("xT", [T, B_LOC], BF16, isOutput=False)
    # per-gate big stationary [K=128, 4, 128] and x/bias stationary [K=2, 4, 128]
    wbig = nc.declare_dram_parameter("wbig", [128, 4 * 128], BF16, isOutput=False)
    wxb = nc.declare_dram_parameter("wxb", [2, 4 * 128], BF16, isOutput=False)
    wlin = nc.declare_dram_parameter("wlin", [H, H], BF16, isOutput=False)
    blin = nc.declare_dram_parameter("blin", [128, H], F32, isOutput=False)
    out = nc.declare_dram_parameter("out", [B_LOC, H], F32, isOutput=True)

    with TileContext(nc) as tc:
        with (
            tc.tile_pool(name="const", bufs=1) as cpool,
            tc.tile_pool(name="state", bufs=1) as spool,
            tc.tile_pool(name="work", bufs=3) as wpool,
            tc.tile_pool(name="psum", bufs=1, space="PSUM") as ppool,
        ):
            wbig_s = cpool.tile([128, 4, 128], BF16, tag="wbig", name="wbig")
            nc.sync.dma_start(wbig_s[:], wbig[:])
            wxb_s = cpool.tile([2, 4, 128], BF16, tag="wxb", name="wxb")
            nc.sync.dma_start(wxb_s[:], wxb[:])
            wlin_s = cpool.tile([H, H], BF16, tag="wlin", name="wlin")
            nc.sync.dma_start(wlin_s[:], wlin[:])
            blin_s = cpool.tile([128, H], F32, tag="blin", name="blin")
            nc.sync.dma_start(blin_s[:], blin[:])

            # State tiles: S = [h0 (0:64); h1 (64:128)], double-buffered.
            S = []
            c_st = []
            h1f = []
            for g in range(NG):
                S.append([spool.tile([2 * H, NB], BF16, tag=f"S{g}_{p}",
                                     name=f"S{g}_{p}") for p in range(2)])
                c_st.append(spool.tile([2 * H, NB], BF16, tag=f"c{g}", name=f"c{g}"))
                h1f.append(spool.tile([H, NB], BF16, tag=f"h1f{g}", name=f"h1f{g}"))
                for p in range(2):
                    nc.gpsimd.memset(S[g][p][:], 0.0)
                nc.gpsimd.memset(c_st[g][:], 0.0)
            # x/ones moving tiles: partition 0 = x_t (whole core batch),
            # partition 1 = ones.  Double-buffered; DMA writes row 0 only.
            X1 = [spool.tile([2, B_LOC], BF16, tag=f"X1_{p}", name=f"X1_{p}")
                  for p in range(2)]
            for p in range(2):
                nc.gpsimd.memset(X1[p][:], 1.0)
            nc.sync.dma_start(X1[0][0:1, :], xT[0:1, :])

            # PSUM: per group, one [128, 3, 512] tile = 3 banks (i,f,o; each
            # gate's data in cols 0:256 of its own bank) + one [128, 512]
            # bank for g.  2 groups x 4 banks = 8 banks.
            ps_ifo = [ppool.tile([2 * H, 3, 512], F32, tag=f"ps_ifo{g}",
                                 name=f"ps_ifo{g}") for g in range(NG)]
            ps_g = [ppool.tile([2 * H, 512], F32, tag=f"ps_g{g}",
                               name=f"ps_g{g}") for g in range(NG)]

            def gate_ps(g, gt):
                if gt == "g":
                    return ps_g[g][:, 0:NB]
                k = {"i": 0, "f": 1, "o": 2}[gt]
                return ps_ifo[g][:, k, 0:NB]

            def step(s):
                cur, nxt = s % 2, (s + 1) % 2
                l0 = s < T
                l1 = s >= 1
                lo = 0 if l0 else H
                hi = 2 * H if l1 else H
                for g in range(NG):
                    cols = slice(g * NB, (g + 1) * NB)
                    # 8 matmuls: per gate, x/bias (K=2, start) + big (K=128,
                    # accumulate).  All x/bias matmuls first: they only need
                    # the x DMA (ready early, run during the DVE tail), and
                    # separating each same-region pair by 3 other matmuls
                    # keeps the PE pipelined (back-to-back same-bank
                    # accumulation stalls on the drain).
                    for gt in GATES:
                        gi = GATES.index(gt)
                        nc.tensor.matmul(gate_ps(g, gt), wxb_s[:, gi, :],
                                         X1[cur][:, cols], start=True, stop=False)
                    for gt in GATES:
                        gi = GATES.index(gt)
                        nc.tensor.matmul(gate_ps(g, gt), wbig_s[:, gi, :],
                                         S[g][cur][:], start=False, stop=True)
                    # Activations: one merged sigmoid (i,f,o), one tanh (g).
                    a = wpool.tile([2 * H, 3, NB], BF16, tag=f"a{g}", name=f"a{g}")
                    nc.scalar.activation(a[lo:hi, :, :],
                                         ps_ifo[g][lo:hi, :, 0:NB], AF.Sigmoid)
                    ag = wpool.tile([2 * H, NB], BF16, tag=f"ag{g}", name=f"ag{g}")
                    nc.scalar.activation(ag[lo:hi, :], ps_g[g][lo:hi, 0:NB], AF.Tanh)
                    # Cell update (VectorE, bf16).
                    t1 = wpool.tile([2 * H, NB], BF16, tag=f"t1{g}", name=f"t1{g}")
                    nc.vector.tensor_tensor(t1[lo:hi, :], a[lo:hi, 1, :],
                                            c_st[g][lo:hi, :], op=OP.mult)
                    t2 = wpool.tile([2 * H, NB], BF16, tag=f"t2{g}", name=f"t2{g}")
                    nc.vector.tensor_tensor(t2[lo:hi, :], a[lo:hi, 0, :],
                                            ag[lo:hi, :], op=OP.mult)
                    nc.vector.tensor_tensor(c_st[g][lo:hi, :], t1[lo:hi, :],
                                            t2[lo:hi, :], op=OP.add)
                    tC = wpool.tile([2 * H, NB], BF16, tag=f"tC{g}", name=f"tC{g}")
                    nc.scalar.activation(tC[lo:hi, :], c_st[g][lo:hi, :], AF.Tanh)
                    if s < T:
                        nc.vector.tensor_tensor(S[g][nxt][lo:hi, :],
                                                a[lo:hi, 2, :], tC[lo:hi, :],
                                                op=OP.mult)
                    else:
                        nc.vector.tensor_tensor(h1f[g][:], a[H:2 * H, 2, :],
                                                tC[H:2 * H, :], op=OP.mult)
                if s + 1 < T:
                    nc.sync.dma_start(X1[nxt][0:1, :], xT[s + 1:s + 2, :])

            for s in range(T + 1):
                step(s)

            # final linear: out[b, :] = h1f.T @ wlin + blin
            for g in range(NG):
                for blk in range(NB // 128):
                    psl = ppool.tile([128, H], F32, tag="ps_ifo0", name="psl")
                    nc.tensor.matmul(psl[:], h1f[g][:, blk * 128:(blk + 1) * 128],
                                     wlin_s[:], start=True, stop=True)
                    ob = wpool.tile([128, H], F32, tag="ob", name="ob")
                    nc.vector.scalar_tensor_tensor(ob[:], psl[:], 1.0,
                                                   blin_s[:],
                                                   op0=OP.mult, op1=OP.add)
                    row0 = g * NB + blk * 128
                    nc.sync.dma_start(out[row0:row0 + 128, :], ob[:])

    _split_excess_waits(nc, limit=1)
    return nc


def _prep_inputs(inputs):
    bf = ml_dtypes.bfloat16
    f32 = np.float32
    recipe = np.ascontiguousarray(np.asarray(inputs["recipe"], f32).reshape(B, T))
    W_hh0 = np.asarray(inputs["W_hh0"], f32)   # [256, 64]
    W_ih0 = np.asarray(inputs["W_ih0"], f32)   # [256, 1]
    W_ih1 = np.asarray(inputs["W_ih1"], f32)   # [256, 64]
    W_hh1 = np.asarray(inputs["W_hh1"], f32)   # [256, 64]
    b0 = np.asarray(inputs["b_ih0"], f32) + np.asarray(inputs["b_hh0"], f32)
    b1 = np.asarray(inputs["b_ih1"], f32) + np.asarray(inputs["b_hh1"], f32)

    # Per-gate big stationary [K=128, M=128]:
    #   cols 0:64 (layer0 out rows): rows 0:64 = W_hh0.T, rows 64:128 = 0
    #   cols 64:128 (layer1 out rows): rows 0:64 = W_ih1.T, 64:128 = W_hh1.T
    wbig = np.zeros((4, 128, 128), f32)
    wxb = np.zeros((4, 2, 128), f32)
    for gi, gt in enumerate(GATES):
        rows = slice(GCOL[gt] * H, (GCOL[gt] + 1) * H)
        wbig[gi, 0:64, 0:64] = W_hh0[rows, :].T
        wbig[gi, 0:64, 64:128] = W_ih1[rows, :].T
        wbig[gi, 64:128, 64:128] = W_hh1[rows, :].T
        wxb[gi, 0, 0:64] = W_ih0[rows, 0]
        wxb[gi, 1, 0:64] = b0[rows]
        wxb[gi, 1, 64:128] = b1[rows]

    wlin = np.ascontiguousarray(np.asarray(inputs["W_lin"], f32).T).astype(bf)
    blin = np.tile(np.asarray(inputs["b_lin"], f32), (128, 1))
    # SBUF layout: partition dim = K first -> [K, gate, M]
    wbig2 = np.ascontiguousarray(wbig.transpose(1, 0, 2)).reshape(128, 512).astype(bf)
    wxb2 = np.ascontiguousarray(wxb.transpose(1, 0, 2)).reshape(2, 512).astype(bf)

    in_maps = []
    for i in range(N_CORES):
        shard = recipe[i * B_LOC:(i + 1) * B_LOC]
        xTs = np.ascontiguousarray(shard.T).astype(bf)
        in_maps.append({"xT": xTs, "wbig": wbig2, "wxb": wxb2,
                        "wlin": wlin, "blin": blin})
    return in_maps


_PROGRAM = []


def _run(inputs, trace=False):
    from concourse.bass_utils import run_bass_kernel_spmd
    if not _PROGRAM:
        _PROGRAM.append(_build_program())
    nc = _PROGRAM[0]
    in_maps = _prep_inputs(inputs)
    last_err = None
    for attempt in range(3):
        try:
            res = run_bass_kernel_spmd(nc, in_maps,
                                       core_ids=list(range(N_CORES)), trace=trace)
            outs = [np.asarray(res.results[i]["out"]) for i in range(N_CORES)]
            return np.concatenate(outs, axis=0), res
        except Exception as e:  # transient first-exec device faults: retry
            last_err = e
    raise last_err


def kernel(**inputs):
    full, _ = _run(inputs, trace=False)
    return full.astype(np.float32)


# revision 8
# speedup vs baseline: 1.5227x; 1.2479x over previous
"""Trainium2 Bass kernel for nn_AIGStateEncoder (2-layer LSTM + linear head).

Data-parallel over batch: B=4096 rows split across 8 NeuronCores (512 each).
Per core the two LSTM layers are fused into one recurrence ("combined step"
s runs layer0 at t=s and layer1 at t=s-1), with the state kept transposed
(hidden units on SBUF partitions, batch on the free dimension) and the two
layers stacked on the 128 partitions: [layer0 (0:64); layer1 (64:128)].
Two independent batch groups of 256 rows hide each other's serial chain.

ScalarE is the bottleneck engine (activation cost ~ (FD + 224c)/1.2GHz,
1x rate), so the design minimizes activation instruction count per step:

  - One state tile S = [h0; h1] (K=128) per group; each gate's preact is ONE
    matmul with a [128, 128] stationary whose columns 0:64 carry layer0's
    W_hh0 (zeros in the h1 rows) and columns 64:128 carry layer1's
    [W_ih1; W_hh1].  A second K=2 matmul accumulates x_t * W_ih0 + biases
    from a tiny [x_t; 1] moving tile, so no activation needs the bias port.
  - Gates i,f,o live in three consecutive PSUM banks (one [128, 3, 512]
    tile); a single SIGMOID activation with a strided [128, 3, 256] access
    pattern covers all three.  Gate g and tanh(c) are separate TANH ops.
  - Cell update on VectorE in bf16: t1=f*c, t2=i*g, c=t1+t2, S'=o*tanh(c)
    (one op writes both h0 and h1 halves).

Everything computes in bf16 (fp32 PSUM accumulation).
"""
import sys

if '/opt/trn_rl_repo' not in sys.path:
    sys.path.insert(0, '/opt/trn_rl_repo')

import numpy as np
import ml_dtypes

B, T, H = 4096, 256, 64
N_CORES = 8
B_LOC = B // N_CORES  # 512

GATES = ("i", "f", "o", "g")  # i,f,o merged sigmoid; g separate tanh
GCOL = {"i": 0, "f": 1, "g": 2, "o": 3}  # PyTorch gate order i,f,g,o


def _split_excess_waits(nc, limit=1):
    """The walrus build in this container accepts at most one sync wait per
    instruction.  Hoist excess waits onto NoOps inserted just before the
    instruction on the same engine (same-engine program order preserves the
    synchronization semantics)."""
    import concourse.mybir as mybir
    ctr = 0
    for f in nc.m.functions:
        for bb in f.blocks:
            il = bb.instructions
            i = 0
            while i < len(il):
                ins = il[i]
                si = ins.sync_info
                if si is not None and si.on_wait and len(si.on_wait) > limit:
                    waits = list(si.on_wait)
                    excess, keep = waits[:-limit], waits[-limit:]
                    while excess:
                        chunk, excess = excess[:limit], excess[limit:]
                        nop = mybir.InstNoOp(name=f"waitsplit_{ctr}", ins=[], outs=[])
                        ctr += 1
                        nop.engine = ins.engine
                        nop.sync_info = mybir.SyncInfo(on_wait=chunk, on_update=[])
                        il.insert(i, nop)
                        i += 1
                    ins.sync_info = mybir.SyncInfo(on_wait=keep,
                                                   on_update=list(si.on_update))
                i += 1


def _build_program(n_groups=2):
    import concourse.bass as bass
    import concourse.mybir as mybir
    from concourse.tile import TileContext

    BF16 = mybir.dt.bfloat16
    F32 = mybir.dt.float32
    AF = mybir.ActivationFunctionType
    OP = mybir.AluOpType

    NG = n_groups
    NB = B_LOC // NG

    nc = bass.Bass()
    # xT2: x transposed, duplicated row layout [T, B_LOC] (one DMA per step)
    xT = nc.declare_dram_parameter("xT", [T, B_LOC], BF16, isOutput=False)
    # per-gate big stationary [K=128, 4, 128] and x/bias stationary [K=2, 4, 128]
    wbig = nc.declare_dram_parameter("wbig", [128, 4 * 128], BF16, isOutput=False)
    wxb = nc.declare_dram_parameter("wxb", [2, 4 * 128], BF16, isOutput=False)
    wlin = nc.declare_dram_parameter("wlin", [H, H], BF16, isOutput=False)
    blin = nc.declare_dram_parameter("blin", [128, H], F32, isOutput=False)
    out = nc.declare_dram_parameter("out", [B_LOC, H], F32, isOutput=True)

    with TileContext(nc) as tc:
        with (
            tc.tile_pool(name="const", bufs=1) as cpool,
            tc.tile_pool(name="state", bufs=1) as spool,
            tc.tile_pool(name="work", bufs=3) as wpool,
            tc.tile_pool(name="psum", bufs=1, space="PSUM") as ppool,
        ):
            wbig_s = cpool.tile([128, 4, 128], BF16, tag="wbig", name="wbig")
            nc.sync.dma_start(wbig_s[:], wbig[:])
            wxb_s = cpool.tile([2, 4, 128], BF16, tag="wxb", name="wxb")
            nc.sync.dma_start(wxb_s[:], wxb[:])
            wlin_s = cpool.tile([H, H], BF16, tag="wlin", name="wlin")
            nc.sync.dma_start(wlin_s[:], wlin[:])
            blin_s = cpool.tile([128, H], F32, tag="blin", name="blin")
            nc.sync.dma_start(blin_s[:], blin[:])

            # State tiles: S = [h0 (0:64); h1 (64:128)], double-buffered.
            S = []
            c_st = []
            h1f = []
            for g in range(NG):
                S.append([spool.tile([2 * H, NB], BF16, tag=f"S{g}_{p}",
                                     name=f"S{g}_{p}") for p in range(2)])
                c_st.append(spool.tile([2 * H, NB], BF16, tag=f"c{g}", name=f"c{g}"))
                h1f.append(spool.tile([H, NB], BF16, tag=f"h1f{g}", name=f"h1f{g}"))
                for p in range(2):
                    nc.gpsimd.memset(S[g][p][:], 0.0)
                nc.gpsimd.memset(c_st[g][:], 0.0)
            # x/ones moving tiles: partition 0 = x_t (whole core batch),
            # partition 1 = ones.  Double-buffered; DMA writes row 0 only.
            X1 = [spool.tile([2, B_LOC], BF16, tag=f"X1_{p}", name=f"X1_{p}")
                  for p in range(2)]
            for p in range(2):
                nc.gpsimd.memset(X1[p][:], 1.0)
            nc.sync.dma_start(X1[0][0:1, :], xT[0:1, :])

            # PSUM: per group, one [128, 3, 512] tile = 3 banks (i,f,o; each
            # gate's data in cols 0:256 of its own bank) + one [128, 512]
            # bank for g.  2 groups x 4 banks = 8 banks.
            ps_ifo = [ppool.tile([2 * H, 3, 512], F32, tag=f"ps_ifo{g}",
                                 name=f"ps_ifo{g}") for g in range(NG)]
            ps_g = [ppool.tile([2 * H, 512], F32, tag=f"ps_g{g}",
                               name=f"ps_g{g}") for g in range(NG)]

            def gate_ps(g, gt):
                if gt == "g":
                    return ps_g[g][:, 0:NB]
                k = {"i": 0, "f": 1, "o": 2}[gt]
                return ps_ifo[g][:, k, 0:NB]

            a_t = {}   # live act tiles per group (head -> tail handoff)

            def rng(s):
                lo = 0 if s < T else H
                hi = 2 * H if s >= 1 else H
                return lo, hi

            def head(g, s):
                """xb+big matmuls, sigmoid(i,f,o), tanh(g), t1/t2/c update."""
                cur = s % 2
                lo, hi = rng(s)
                cols = slice(g * NB, (g + 1) * NB)
                # 8 matmuls: per gate, x/bias (K=2, start) + big (K=128,
                # accumulate).  All x/bias matmuls first: they only need the
                # x DMA (ready early, run during the other group's tail), and
                # separating each same-region pair by 3 other matmuls keeps
                # the PE pipelined (back-to-back same-bank accumulation
                # stalls on the drain).
                for gt in GATES:
                    gi = GATES.index(gt)
                    nc.tensor.matmul(gate_ps(g, gt), wxb_s[:, gi, :],
                                     X1[cur][:, cols], start=True, stop=False)
                for gt in GATES:
                    gi = GATES.index(gt)
                    nc.tensor.matmul(gate_ps(g, gt), wbig_s[:, gi, :],
                                     S[g][cur][:], start=False, stop=True)
                # Activations: one merged sigmoid (i,f,o), one tanh (g).
                a = wpool.tile([2 * H, 3, NB], BF16, tag=f"a{g}", name=f"a{g}")
                nc.scalar.activation(a[lo:hi, :, :],
                                     ps_ifo[g][lo:hi, :, 0:NB], AF.Sigmoid)
                ag = wpool.tile([2 * H, NB], BF16, tag=f"ag{g}", name=f"ag{g}")
                nc.scalar.activation(ag[lo:hi, :], ps_g[g][lo:hi, 0:NB], AF.Tanh)
                # Cell update (VectorE, bf16).
                t1 = wpool.tile([2 * H, NB], BF16, tag=f"t1{g}", name=f"t1{g}")
                nc.vector.tensor_tensor(t1[lo:hi, :], a[lo:hi, 1, :],
                                        c_st[g][lo:hi, :], op=OP.mult)
                t2 = wpool.tile([2 * H, NB], BF16, tag=f"t2{g}", name=f"t2{g}")
                nc.vector.tensor_tensor(t2[lo:hi, :], a[lo:hi, 0, :],
                                        ag[lo:hi, :], op=OP.mult)
                nc.vector.tensor_tensor(c_st[g][lo:hi, :], t1[lo:hi, :],
                                        t2[lo:hi, :], op=OP.add)
                a_t[g] = a

            def tail(g, s):
                """tanh(c) and the h-state write for step s."""
                nxt = (s + 1) % 2
                lo, hi = rng(s)
                a = a_t[g]
                tC = wpool.tile([2 * H, NB], BF16, tag=f"tC{g}", name=f"tC{g}")
                nc.scalar.activation(tC[lo:hi, :], c_st[g][lo:hi, :], AF.Tanh)
                if s < T:
                    nc.vector.tensor_tensor(S[g][nxt][lo:hi, :],
                                            a[lo:hi, 2, :], tC[lo:hi, :],
                                            op=OP.mult)
                else:
                    nc.vector.tensor_tensor(h1f[g][:], a[H:2 * H, 2, :],
                                            tC[H:2 * H, :], op=OP.mult)

            # Antiphase emission: the scalar-queue order per half-step is
            # [tail(other), sig(this), tanh_g(this)], so each group's
            # c-dependent tanh never blocks the other group's gate work.
            for s in range(T + 1):
                if s + 1 < T:
                    nc.sync.dma_start(X1[(s + 1) % 2][0:1, :], xT[s + 1:s + 2, :])
                head(0, s)
                if s >= 1:
                    tail(1, s - 1)
                head(1, s)
                tail(0, s)
            tail(1, T)

            # final linear: out[b, :] = h1f.T @ wlin + blin
            for g in range(NG):
                for blk in range(NB // 128):
                    psl = ppool.tile([128, H], F32, tag="ps_ifo0", name="psl")
                    nc.tensor.matmul(psl[:], h1f[g][:, blk * 128:(blk + 1) * 128],
                                     wlin_s[:], start=True, stop=True)
                    ob = wpool.tile([128, H], F32, tag="ob", name="ob")
                    nc.vector.scalar_tensor_tensor(ob[:], psl[:], 1.0,
                                                   blin_s[:],
                                                   op0=OP.mult, op1=OP.add)
                    row0 = g * NB + blk * 128
                    nc.sync.dma_start(out[row0:row0 + 128, :], ob[:])

    _split_excess_waits(nc, limit=1)
    return nc


def _prep_inputs(inputs):
    bf = ml_dtypes.bfloat16
    f32 = np.float32
    recipe = np.ascontiguousarray(np.asarray(inputs["recipe"], f32).reshape(B, T))
    W_hh0 = np.asarray(inputs["W_hh0"], f32)   # [256, 64]
    W_ih0 = np.asarray(inputs["W_ih0"], f32)   # [256, 1]
    W_ih1 = np.asarray(inputs["W_ih1"], f32)   # [256, 64]
    W_hh1 = np.asarray(inputs["W_hh1"], f32)   # [256, 64]
    b0 = np.asarray(inputs["b_ih0"], f32) + np.asarray(inputs["b_hh0"], f32)
    b1 = np.asarray(inputs["b_ih1"], f32) + np.asarray(inputs["b_hh1"], f32)

    # Per-gate big stationary [K=128, M=128]:
    #   cols 0:64 (layer0 out rows): rows 0:64 = W_hh0.T, rows 64:128 = 0
    #   cols 64:128 (layer1 out rows): rows 0:64 = W_ih1.T, 64:128 = W_hh1.T
    wbig = np.zeros((4, 128, 128), f32)
    wxb = np.zeros((4, 2, 128), f32)
    for gi, gt in enumerate(GATES):
        rows = slice(GCOL[gt] * H, (GCOL[gt] + 1) * H)
        wbig[gi, 0:64, 0:64] = W_hh0[rows, :].T
        wbig[gi, 0:64, 64:128] = W_ih1[rows, :].T
        wbig[gi, 64:128, 64:128] = W_hh1[rows, :].T
        wxb[gi, 0, 0:64] = W_ih0[rows, 0]
        wxb[gi, 1, 0:64] = b0[rows]
        wxb[gi, 1, 64:128] = b1[rows]

    wlin = np.ascontiguousarray(np.asarray(inputs["W_lin"], f32).T).astype(bf)
    blin = np.tile(np.asarray(inputs["b_lin"], f32), (128, 1))
    # SBUF layout: partition dim = K first -> [K, gate, M]
    wbig2 = np.ascontiguousarray(wbig.transpose(1, 0, 2)).reshape(128, 512).astype(bf)
    wxb2 = np.ascontiguousarray(wxb.transpose(1, 0, 2)).reshape(2, 512).astype(bf)

    in_maps = []
    for i in range(N_CORES):
        shard = recipe[i * B_LOC:(i + 1) * B_LOC]
        xTs = np.ascontiguousarray(shard.T).astype(bf)
        in_maps.append({"xT": xTs, "wbig": wbig2, "wxb": wxb2,
                        "wlin": wlin, "blin": blin})
    return in_maps


_PROGRAM = []


def _run(inputs, trace=False):
    from concourse.bass_utils import run_bass_kernel_spmd
    if not _PROGRAM:
        _PROGRAM.append(_build_program())
    nc = _PROGRAM[0]
    in_maps = _prep_inputs(inputs)
    last_err = None
    for attempt in range(3):
        try:
            res = run_bass_kernel_spmd(nc, in_maps,
                                       core_ids=list(range(N_CORES)), trace=trace)
            outs = [np.asarray(res.results[i]["out"]) for i in range(N_CORES)]
            return np.concatenate(outs, axis=0), res
        except Exception as e:  # transient first-exec device faults: retry
            last_err = e
    raise last_err


def kernel(**inputs):
    full, _ = _run(inputs, trace=False)
    return full.astype(np.float32)
